# revision 19
# baseline (speedup 1.0000x reference)
"""Trainium2 Bass kernel for nn_ReaReaConv (GCN-style message passing with
dynamic edge gating).

Math (per batch b):
    deg[n]   = in-degree(n) + 1 (self loop);  dis = rsqrt(deg)
    f_e      = keep*fdo + (1-keep)*(1-fdo), keep = sigmoid(2*flux[src]*flux[tgt])
    out[t]   = dis_t * ( T[t] @ Wc^T + V[t] @ (Wd-Wc)^T ) + bias
    T[t]     = sum_{e->t} dis_src * x[src_e] + dis_t * x[t]   (self loop f=0)
    V[t]     = sum_{e->t} dis_src * f_e * x[src_e]

Sharding: each of the 8 cores owns N/8 target nodes (tiles of 125). Host sorts
edges by target tile, prescales the gather table by dis_src and casts to bf16
(index/layout/elementwise-prescale only; per-edge math runs on device).

v4 design notes:
 - Gather table rows are batch-PAIR-interleaved ([x0[c],x1[c]] pairs) so the
   on-device f-scale multiply qualifies for the DVE 2x 16-bit mode (the f
   broadcast lands mid-dim, innermost is the contiguous batch pair).
 - One-hot is built t-major (col = t*ct + c) against a host-built wide iota so
   every operand of the is_equal has innermost stride 1 -> DVE 2x mode.
 - dma_gather uses prepare_only + trigger_dma so descriptor generation
   pipelines with DMA drain instead of serializing on completion.
 - Self-loop contributions are streamed contiguously (no gather descriptors)
   and added to T as a [125, 128] DVE add.
 - Per 128-edge chunk: ONE matmul, lhsT = one-hot [128, 125], rhs = [y | f*y]
   [128, 256] accumulating PSUM [125, 256] = [T | V]. Epilogue PE-transposes
   (deinterleaving batch pairs for free) and projects with Wc / (Wd-Wc).
"""

from dataclasses import dataclass

import ml_dtypes
import numpy as np

BF16 = ml_dtypes.bfloat16

# -------------------- problem constants --------------------
N_NODES = 50000
N_EDGES = 1600000
BATCH = 2
C = 64
N_CORES = 8
TILE = 125           # target nodes per tile (one-hot width)
CHUNK = 128          # edges per matmul chunk (PE contraction)
SPLIT = 32768        # gather-table split (int16 signed index limit)


@dataclass(frozen=True)
class Cfg:
    n_nodes: int
    n_cores: int
    tile: int
    split: int
    capa: int  # chunks per tile from table A (src < split)
    capb: int  # chunks per tile from table B

    @property
    def nodes_per_core(self):
        return self.n_nodes // self.n_cores

    @property
    def ntc(self):  # tiles per core
        return self.nodes_per_core // self.tile

    @property
    def ct(self):
        return self.capa + self.capb

    @property
    def ctn(self):
        return self.ntc * self.ct

    @property
    def na(self):
        return min(self.split, self.n_nodes)

    @property
    def nb(self):
        return self.n_nodes - self.na


# -------------------- host prep (indices / layout / dtype cast) ------------

def _wrap16(idx_flat):
    """dma_gather index layout: [128, n/16] int16, idx[p, s] = flat[s*16+p],
    replicated across the 8 gpsimd cores (partition blocks of 16)."""
    n = len(idx_flat)
    assert n % 16 == 0
    w = np.asarray(idx_flat, np.int16).reshape(n // 16, 16).T  # [16, n/16]
    return np.tile(w, (8, 1))  # [128, n/16]


def prep(x, edge_index, f_disc_orig, fluxes, cfg: Cfg):
    """Returns (shared dict, list of per-core dicts). Index/layout/cast and
    elementwise dis prescale only — per-edge math runs on device."""
    n = cfg.n_nodes
    src0 = np.asarray(edge_index[0]).astype(np.int64)
    tgt0 = np.asarray(edge_index[1]).astype(np.int64)
    x = np.asarray(x, np.float32)
    fdo_in = np.asarray(f_disc_orig, np.float32)
    fluxes = np.asarray(fluxes, np.float32)

    deg = (np.bincount(tgt0, minlength=n) + 1).astype(np.float32)  # int-valued
    dis = 1.0 / np.sqrt(deg)

    # pair-interleaved, dis-prescaled bf16 gather table:
    # row n = [x0[n,0], x1[n,0], x0[n,1], x1[n,1], ...] * dis_n
    xi = np.empty((n, 2 * C), np.float32)
    xi[:, 0::2] = x[0] * dis[:, None]
    xi[:, 1::2] = x[1] * dis[:, None]
    xi = xi.astype(BF16)

    per_edge_all = np.stack([
        fdo_in,
        fluxes[0][src0],
        fluxes[1][src0],
        fluxes[0][tgt0],
        fluxes[1][tgt0],
    ])  # [5, E]: fdo, fs0, fs1, ft0, ft1   (no self loops)

    perm = np.argsort(tgt0, kind="stable")
    src_s = src0[perm]
    tgt_s = tgt0[perm]
    per_edge = per_edge_all[:, perm]

    tile_starts = np.searchsorted(tgt_s, np.arange(0, n + 1, cfg.tile))
    is_a = src_s < cfg.split

    ct, ntc, ctn = cfg.ct, cfg.ntc, cfg.ctn
    T = cfg.tile

    shared = {
        "xpa": np.ascontiguousarray(xi[: cfg.na]),
        "xpb": np.ascontiguousarray(xi[cfg.na:]),
        # wide iota, t-major: col t*ct + c = t
        "iotaw": np.tile(np.repeat(np.arange(T, dtype=np.float32), ct),
                         (128, 1)).astype(BF16),
        "ident": np.eye(128, dtype=np.float32).astype(BF16),
    }

    names = ["fdo", "fs0", "fs1", "ft0", "ft1"]
    npc = cfg.nodes_per_core
    cores = []
    for core in range(cfg.n_cores):
        tl_all = np.full((128, ctn), -1.0, np.float32)
        pe_all = np.zeros((5, 128, ctn), np.float32)
        idx16 = np.zeros((128, ctn * 8), np.int16)
        degown = np.ones((128, ntc), np.float32)
        for tt in range(ntc):
            t = core * ntc + tt
            t0 = t * cfg.tile
            s, e = tile_starts[t], tile_starts[t + 1]
            sel_a = np.nonzero(is_a[s:e])[0] + s
            sel_b = np.nonzero(~is_a[s:e])[0] + s
            nA, nB = len(sel_a), len(sel_b)
            assert nA <= cfg.capa * CHUNK, (t, nA, cfg.capa * CHUNK)
            assert nB <= cfg.capb * CHUNK, (t, nB, cfg.capb * CHUNK)

            ids = np.zeros(ct * CHUNK, np.int64)
            tl = np.full(ct * CHUNK, -1.0, np.float32)
            pe = np.zeros((5, ct * CHUNK), np.float32)
            off = cfg.capa * CHUNK
            ids[:nA] = src_s[sel_a]
            ids[off:off + nB] = src_s[sel_b] - cfg.na
            tl[:nA] = tgt_s[sel_a] - t0
            tl[off:off + nB] = tgt_s[sel_b] - t0
            pe[:, :nA] = per_edge[:, sel_a]
            pe[:, off:off + nB] = per_edge[:, sel_b]

            # chunk-transposed layout: element (p, c) = edge[c*128+p]
            cols = slice(tt * ct, (tt + 1) * ct)
            tl_all[:, cols] = tl.reshape(ct, CHUNK).T
            for j in range(5):
                pe_all[j][:, cols] = pe[j].reshape(ct, CHUNK).T

            islice = idx16[:, tt * ct * 8:(tt + 1) * ct * 8]
            islice[:, : cfg.capa * 8] = _wrap16(ids[:off])
            islice[:, cfg.capa * 8:] = _wrap16(ids[off:])

            degown[:cfg.tile, tt] = deg[t0:t0 + cfg.tile]
        d = {"tl": tl_all.astype(BF16), "idx16": idx16, "degown": degown,
             "yloop": np.ascontiguousarray(xi[core * npc:(core + 1) * npc])}
        for j, nm in enumerate(names):
            d[nm] = np.ascontiguousarray(pe_all[j])
        cores.append(d)
    return shared, cores


# -------------------- device program --------------------

def build_nc(cfg: Cfg):
    import concourse.bass as bass
    import concourse.tile as tile
    from concourse import bacc, mybir

    dt = mybir.dt
    act = mybir.ActivationFunctionType
    alu = mybir.AluOpType

    ct, capa, capb = cfg.ct, cfg.capa, cfg.capb
    ntc, T, ctn = cfg.ntc, cfg.tile, cfg.ctn

    nc = bacc.Bacc("TRN2", target_bir_lowering=False, debug=False,
                   num_swdge_queues=4)

    xpa = nc.dram_tensor("xpa", [cfg.na, 2 * C], dt.bfloat16, kind="ExternalInput")
    xpb = nc.dram_tensor("xpb", [cfg.nb, 2 * C], dt.bfloat16, kind="ExternalInput")
    yl_d = nc.dram_tensor("yloop", [ntc * T, 2 * C], dt.bfloat16, kind="ExternalInput")
    tl_d = nc.dram_tensor("tl", [128, ctn], dt.bfloat16, kind="ExternalInput")
    fdo_d = nc.dram_tensor("fdo", [128, ctn], dt.float32, kind="ExternalInput")
    fs0_d = nc.dram_tensor("fs0", [128, ctn], dt.float32, kind="ExternalInput")
    fs1_d = nc.dram_tensor("fs1", [128, ctn], dt.float32, kind="ExternalInput")
    ft0_d = nc.dram_tensor("ft0", [128, ctn], dt.float32, kind="ExternalInput")
    ft1_d = nc.dram_tensor("ft1", [128, ctn], dt.float32, kind="ExternalInput")
    idx16_d = nc.dram_tensor("idx16", [128, ctn * 8], dt.int16, kind="ExternalInput")
    degown_d = nc.dram_tensor("degown", [128, ntc], dt.float32, kind="ExternalInput")
    iotaw_d = nc.dram_tensor("iotaw", [128, T * ct], dt.bfloat16, kind="ExternalInput")
    ident_d = nc.dram_tensor("ident", [128, 128], dt.bfloat16, kind="ExternalInput")
    wct_d = nc.dram_tensor("wct2", [128, C], dt.bfloat16, kind="ExternalInput")
    wdt_d = nc.dram_tensor("wdt2", [128, C], dt.bfloat16, kind="ExternalInput")
    bias_d = nc.dram_tensor("biasr", [128, C], dt.float32, kind="ExternalInput")
    out0 = nc.dram_tensor("out0", [ntc * T, C], dt.float32, kind="ExternalOutput")
    out1 = nc.dram_tensor("out1", [ntc * T, C], dt.float32, kind="ExternalOutput")
    outs = [out0, out1]

    with tile.TileContext(nc) as tc:
        with (
            tc.tile_pool(name="const", bufs=1) as constp,
            tc.tile_pool(name="res", bufs=1) as resp,
        ):
            iotaw_sb = constp.tile([128, T * ct], dt.bfloat16)
            nc.sync.dma_start(iotaw_sb[:], iotaw_d[:, :])
            ident_sb = constp.tile([128, 128], dt.bfloat16)
            nc.sync.dma_start(ident_sb[:], ident_d[:, :])
            bias_sb = constp.tile([128, C], dt.float32)
            nc.sync.dma_start(bias_sb[:], bias_d[:, :])
            wct_sb = constp.tile([128, C], dt.bfloat16)
            nc.sync.dma_start(wct_sb[:], wct_d[:, :])
            wdt_sb = constp.tile([128, C], dt.bfloat16)
            nc.sync.dma_start(wdt_sb[:], wdt_d[:, :])
            # wdc = Wd - Wc (projection uses T*Wc + V*(Wd-Wc))
            wdc_sb = constp.tile([128, C], dt.bfloat16)
            nc.vector.tensor_tensor(wdc_sb[:], wdt_sb[:], wct_sb[:],
                                    alu.subtract)

            # resident per-core data
            tl_sb = resp.tile([128, ctn], dt.bfloat16)
            nc.sync.dma_start(tl_sb[:], tl_d[:, :])
            idx_sb = resp.tile([128, ctn * 8], dt.int16)
            nc.sync.dma_start(idx_sb[:], idx16_d[:, :])
            # f for both batches, interleaved [128, ctn, 2] bf16
            fcat_sb = resp.tile([128, ctn * 2], dt.bfloat16)
            fcat3 = fcat_sb[:].rearrange("p (n b) -> p n b", b=2)

            # dis for own target nodes (fp32, output scale)
            disown_sb = resp.tile([128, ntc], dt.float32)
            nc.sync.dma_start(disown_sb[:], degown_d[:, :])
            nc.vector.reciprocal(disown_sb[:], disown_sb[:])
            nc.scalar.activation(disown_sb[:], disown_sb[:], act.Sqrt)

            # ---- prepass: f0/f1 from flux/fdo ----
            with tc.tile_pool(name="pp", bufs=1) as ppp:
                fdo_sb = ppp.tile([128, ctn], dt.float32)
                nc.sync.dma_start(fdo_sb[:], fdo_d[:, :])
                c1 = ppp.tile([128, ctn], dt.float32)
                nc.vector.tensor_scalar(
                    c1[:], fdo_sb[:], 2.0, -1.0, alu.mult, alu.add)
                c0 = ppp.tile([128, ctn], dt.float32)
                nc.vector.tensor_scalar(
                    c0[:], fdo_sb[:], -1.0, 1.0, alu.mult, alu.add)
                for b, (fsd, ftd) in enumerate(((fs0_d, ft0_d), (fs1_d, ft1_d))):
                    fs_sb = ppp.tile([128, ctn], dt.float32, tag="fs")
                    nc.sync.dma_start(fs_sb[:], fsd[:, :])
                    ft_sb = ppp.tile([128, ctn], dt.float32, tag="ft")
                    nc.sync.dma_start(ft_sb[:], ftd[:, :])
                    nc.vector.tensor_mul(fs_sb[:], fs_sb[:], ft_sb[:])
                    nc.scalar.activation(
                        ft_sb[:], fs_sb[:], act.Sigmoid, scale=2.0)
                    nc.vector.tensor_mul(fs_sb[:], ft_sb[:], c1[:])
                    nc.vector.tensor_add(fcat3[:, :, b], fs_sb[:], c0[:])

            qsems = [nc.alloc_semaphore(f"swdge_q{q}") for q in range(4)]
            for q in range(4):
                nc.gpsimd.sem_clear(qsems[q])

            # ---- main loop over node tiles ----
            with (
                tc.tile_pool(name="xg", bufs=3) as xgp,
                tc.tile_pool(name="yl", bufs=2) as ylp,
                tc.tile_pool(name="oe", bufs=2) as oep,
                tc.tile_pool(name="tvs", bufs=2) as tvsp,
                tc.tile_pool(name="trs", bufs=2) as trsp,
                tc.tile_pool(name="outp", bufs=2) as outsp,
                tc.tile_pool(name="ps_tv", bufs=2, space="PSUM") as pstv,
                tc.tile_pool(name="ps_tr", bufs=2, space="PSUM") as pstr,
                tc.tile_pool(name="ps_o", bufs=2, space="PSUM") as pso,
            ):
                for tt in range(ntc):
                    ib = tt * ct * 8
                    # gathered rows (first half) | f-scaled rows (second half)
                    xg = xgp.tile([128, 2 * ct * 2 * C], dt.bfloat16, tag="xg")
                    xg3 = xg[:].rearrange("p (c r) -> p c r", r=2 * C)
                    ca1 = capa // 2
                    cb1 = capb // 2
                    gathers = [
                        (0, ca1, xpa, 0),
                        (ca1, capa, xpa, 1),
                        (capa, capa + cb1, xpb, 2),
                        (capa + cb1, ct, xpb, 3),
                    ]
                    for (c0_, c1_, tab, q) in gathers:
                        nch = c1_ - c0_
                        nc.gpsimd.dma_gather(
                            xg3[:, c0_:c1_],
                            tab[:, :],
                            idx_sb[:, ib + c0_ * 8: ib + c1_ * 8],
                            nch * CHUNK, nch * CHUNK, 2 * C,
                            single_packet=False, queue_num=q,
                            prepare_only=True, sem=qsems[q],
                        )
                    for q in range(4):
                        nc.gpsimd.trigger_dma(count=None, queue_num=q)

                    # self-loop rows for this tile (contiguous stream)
                    yl_sb = ylp.tile([128, 2 * C], dt.bfloat16, tag="yl")
                    nc.sync.dma_start(
                        yl_sb[:T, :], yl_d[tt * T:(tt + 1) * T, :])

                    # one-hot, t-major (col = t*ct + c); table is
                    # dis_src-prescaled so values are pure 0/1
                    o_all = oep.tile([128, T * ct], dt.bfloat16, tag="oe")
                    o3 = o_all[:].rearrange("p (t c) -> p t c", c=ct)
                    tl_cols = tl_sb[:, tt * ct:(tt + 1) * ct].unsqueeze(1)
                    nc.vector.tensor_tensor(
                        o3,
                        tl_cols.to_broadcast([128, T, ct]),
                        iotaw_sb[:].rearrange("p (t c) -> p t c", c=ct),
                        alu.is_equal,
                    )

                    # gathered data landed (16 engine-completions per prep)
                    for q in range(4):
                        nc.vector.wait_ge(qsems[q], 16 * (tt + 1))

                    # second half = f * first half (per batch, pair-interleaved)
                    xh4 = xg[:, 0:ct * 2 * C].rearrange(
                        "p (c f b) -> p c f b", f=C, b=2)
                    fxh4 = xg[:, ct * 2 * C:2 * ct * 2 * C].rearrange(
                        "p (c f b) -> p c f b", f=C, b=2)
                    fcols = fcat3[:, tt * ct:(tt + 1) * ct, :].unsqueeze(2)
                    nc.vector.tensor_tensor(
                        fxh4, xh4, fcols.to_broadcast([128, ct, C, 2]),
                        alu.mult)

                    # [T | V] accumulation: one matmul per chunk, 256-wide rhs
                    tv_ps = pstv.tile([T, 4 * C], dt.float32, tag="tv")
                    xgv = xg[:].rearrange("p (h w) -> p h w", h=2)
                    for c in range(ct):
                        nc.tensor.matmul(
                            out=tv_ps[:],
                            lhsT=o3[:, :, c],
                            rhs=xgv[:, :, c * 2 * C:(c + 1) * 2 * C],
                            start=(c == 0), stop=(c == ct - 1),
                        )

                    # epilogue: deinterleaving bf16 copy (pair-interleaved ->
                    # batch-major) + self-loop add, transposes to [ch, t],
                    # project
                    tv_sb = tvsp.tile([T, 4 * C], dt.bfloat16, tag="tv_sb")
                    nc.scalar.activation(
                        tv_sb[:].rearrange("p (g b f) -> p g b f",
                                           g=2, b=2, f=C),
                        tv_ps[:].rearrange("p (g f b) -> p g b f",
                                           g=2, f=C, b=2),
                        act.Copy)
                    nc.vector.tensor_add(
                        tv_sb[:, 0:2 * C].rearrange("p (b f) -> p b f", b=2),
                        tv_sb[:, 0:2 * C].rearrange("p (b f) -> p b f", b=2),
                        yl_sb[:T, :].rearrange("p (f b) -> p b f", b=2))
                    tr_ps = pstr.tile([128, 256], dt.bfloat16, tag="tr")
                    nc.tensor.transpose(
                        tr_ps[:, 0:T], tv_sb[:, 0:2 * C], ident_sb[:T, :T])
                    nc.tensor.transpose(
                        tr_ps[:, 128:128 + T], tv_sb[:, 2 * C:4 * C],
                        ident_sb[:T, :T])
                    tr_sb = trsp.tile([128, 256], dt.bfloat16, tag="tr_sb")
                    nc.vector.tensor_copy(
                        out=tr_sb[:, 0:T], in_=tr_ps[:, 0:T])
                    nc.scalar.activation(
                        tr_sb[:, 128:128 + T], tr_ps[:, 128:128 + T], act.Copy)

                    for bi in range(2):
                        rows = slice(64 * bi, 64 * bi + 64)
                        op_ps = pso.tile([T, C], dt.float32, tag=f"op{bi}")
                        nc.tensor.matmul(
                            out=op_ps[:], lhsT=tr_sb[rows, 0:T],
                            rhs=wct_sb[rows, :],
                            start=True, stop=False,
                        )
                        nc.tensor.matmul(
                            out=op_ps[:], lhsT=tr_sb[rows, 128:128 + T],
                            rhs=wdc_sb[rows, :],
                            start=False, stop=True,
                        )
                        o_sb = outsp.tile([128, C], dt.float32, tag=f"os{bi}")
                        nc.vector.tensor_scalar(
                            o_sb[:T, :], op_ps[:], disown_sb[:T, tt:tt + 1],
                            None, alu.mult)
                        nc.vector.tensor_add(
                            o_sb[:T, :], o_sb[:T, :], bias_sb[:T, :])
                        nc.sync.dma_start(
                            outs[bi][tt * T:(tt + 1) * T, :], o_sb[:T, :])

    nc.compile()
    return nc


def _shared_weights(W_conc, W_disc, bias):
    wct2 = np.zeros((128, C), np.float32)
    wdt2 = np.zeros((128, C), np.float32)
    wct2[:64] = np.asarray(W_conc, np.float32).T  # WcT[i, o] = Wc[o, i]
    wct2[64:] = wct2[:64]
    wdt2[:64] = np.asarray(W_disc, np.float32).T
    wdt2[64:] = wdt2[:64]
    biasr = np.tile(np.asarray(bias, np.float32)[None, :], (128, 1))
    return wct2.astype(BF16), wdt2.astype(BF16), biasr


_NC_CACHE = {}


def _caps_needed(edge_index, n, n_cores, tile, split):
    """Max per-tile chunk counts for the A/B table split (no self loops)."""
    src0 = np.asarray(edge_index[0]).astype(np.int64)
    tgt0 = np.asarray(edge_index[1]).astype(np.int64)
    order = np.argsort(tgt0, kind="stable")
    tgt_s, src_s = tgt0[order], src0[order]
    starts = np.searchsorted(tgt_s, np.arange(0, n + 1, tile))
    na = np.add.reduceat((src_s < split).astype(np.int64), starts[:-1])
    tot = np.diff(starts)
    maxa = int(na.max())
    maxb = int((tot - na).max())
    return -(-maxa // CHUNK), -(-maxb // CHUNK)


def _make_in_maps(x, edge_index, f_disc_orig, fluxes, W_conc, W_disc, bias,
                  cfg):
    shared, cores = prep(x, edge_index, f_disc_orig, fluxes, cfg)
    wct2, wdt2, biasr = _shared_weights(W_conc, W_disc, bias)
    in_maps = []
    for core in range(cfg.n_cores):
        m = dict(shared)
        m.update(cores[core])
        m["wct2"] = wct2
        m["wdt2"] = wdt2
        m["biasr"] = biasr
        in_maps.append(m)
    return in_maps


def _run(inputs, trace=False):
    from concourse.bass_utils import run_bass_kernel_spmd

    x = np.asarray(inputs["x"], np.float32)
    n = x.shape[1]
    capa, capb = _caps_needed(inputs["edge_index"], n, N_CORES, TILE, SPLIT)
    cfg = Cfg(n_nodes=n, n_cores=N_CORES, tile=TILE, split=SPLIT,
              capa=max(capa, 2), capb=max(capb, 2))
    in_maps = _make_in_maps(
        x, inputs["edge_index"], inputs["f_disc_orig"], inputs["fluxes"],
        inputs["W_conc"], inputs["W_disc"], inputs["bias"], cfg)

    if cfg not in _NC_CACHE:
        _NC_CACHE[cfg] = build_nc(cfg)
    nc = _NC_CACHE[cfg]

    res = run_bass_kernel_spmd(nc, in_maps, list(range(cfg.n_cores)),
                               trace=trace)
    out = np.zeros((BATCH, n, C), np.float32)
    npc = cfg.nodes_per_core
    for core in range(cfg.n_cores):
        out[0, core * npc:(core + 1) * npc] = res.results[core]["out0"]
        out[1, core * npc:(core + 1) * npc] = res.results[core]["out1"]
    return out, res


def kernel(x, edge_index, f_disc_orig, fluxes, W_conc, W_disc, bias):
    out, _ = _run(dict(x=x, edge_index=edge_index, f_disc_orig=f_disc_orig,
                       fluxes=fluxes, W_conc=W_conc, W_disc=W_disc, bias=bias))
    return out


def profile_run(inputs):
    out, res = _run(inputs, trace=True)
    return res.exec_time_ns


# revision 21
# speedup vs baseline: 1.3418x; 1.3418x over previous
"""Trainium2 Bass kernel for nn_ReaReaConv (GCN-style message passing with
dynamic edge gating).

Math (per batch b):
    deg[n]   = in-degree(n) + 1 (self loop);  dis = rsqrt(deg)
    f_e      = keep*fdo + (1-keep)*(1-fdo), keep = sigmoid(2*flux[src]*flux[tgt])
    out[t]   = dis_t * ( T[t] @ Wc^T + V[t] @ (Wd-Wc)^T ) + bias
    T[t]     = sum_{e->t} dis_src * x[src_e] + dis_t * x[t]   (self loop f=0)
    V[t]     = sum_{e->t} dis_src * f_e * x[src_e]

Sharding: each of the 8 cores owns N/8 target nodes (tiles of 125). Host sorts
edges by target tile, prescales the gather table by dis_src and casts to bf16
(index/layout/elementwise-prescale only; per-edge math runs on device).

v4 design notes:
 - Gather table rows are batch-PAIR-interleaved ([x0[c],x1[c]] pairs) so the
   on-device f-scale multiply qualifies for the DVE 2x 16-bit mode (the f
   broadcast lands mid-dim, innermost is the contiguous batch pair).
 - One-hot is built t-major (col = t*ct + c) against a host-built wide iota so
   every operand of the is_equal has innermost stride 1 -> DVE 2x mode.
 - Self-loop contributions are streamed contiguously (no gather descriptors)
   and added to T as a [125, 128] DVE add.
 - Per 128-edge chunk: ONE matmul, lhsT = one-hot [128, 125], rhs = [y | f*y]
   [128, 256] accumulating PSUM [125, 256] = [T | V]. Epilogue PE-transposes
   (deinterleaving batch pairs for free) and projects with Wc / (Wd-Wc).
"""

from dataclasses import dataclass

import ml_dtypes
import numpy as np

BF16 = ml_dtypes.bfloat16

# -------------------- problem constants --------------------
N_NODES = 50000
N_EDGES = 1600000
BATCH = 2
C = 64
N_CORES = 8
TILE = 125           # target nodes per tile (one-hot width)
CHUNK = 128          # edges per matmul chunk (PE contraction)
SPLIT = 32768        # gather-table split (int16 signed index limit)


@dataclass(frozen=True)
class Cfg:
    n_nodes: int
    n_cores: int
    tile: int
    split: int
    capa: int  # chunks per tile from table A (src < split)
    capb: int  # chunks per tile from table B

    @property
    def nodes_per_core(self):
        return self.n_nodes // self.n_cores

    @property
    def ntc(self):  # tiles per core
        return self.nodes_per_core // self.tile

    @property
    def ct(self):
        return self.capa + self.capb

    @property
    def ctn(self):
        return self.ntc * self.ct

    @property
    def na(self):
        return min(self.split, self.n_nodes)

    @property
    def nb(self):
        return self.n_nodes - self.na


# -------------------- host prep (indices / layout / dtype cast) ------------

def _wrap16(idx_flat):
    """dma_gather index layout: [128, n/16] int16, idx[p, s] = flat[s*16+p],
    replicated across the 8 gpsimd cores (partition blocks of 16)."""
    n = len(idx_flat)
    assert n % 16 == 0
    w = np.asarray(idx_flat, np.int16).reshape(n // 16, 16).T  # [16, n/16]
    return np.tile(w, (8, 1))  # [128, n/16]


def prep(x, edge_index, f_disc_orig, fluxes, cfg: Cfg):
    """Returns (shared dict, list of per-core dicts). Index/layout/cast and
    elementwise dis prescale only — per-edge math runs on device."""
    n = cfg.n_nodes
    src0 = np.asarray(edge_index[0]).astype(np.int64)
    tgt0 = np.asarray(edge_index[1]).astype(np.int64)
    x = np.asarray(x, np.float32)
    fdo_in = np.asarray(f_disc_orig, np.float32)
    fluxes = np.asarray(fluxes, np.float32)

    deg = (np.bincount(tgt0, minlength=n) + 1).astype(np.float32)  # int-valued
    dis = 1.0 / np.sqrt(deg)

    # pair-interleaved, dis-prescaled bf16 gather table:
    # row n = [x0[n,0], x1[n,0], x0[n,1], x1[n,1], ...] * dis_n
    xi = np.empty((n, 2 * C), np.float32)
    xi[:, 0::2] = x[0] * dis[:, None]
    xi[:, 1::2] = x[1] * dis[:, None]
    xi = xi.astype(BF16)

    per_edge_all = np.stack([
        fdo_in,
        fluxes[0][src0],
        fluxes[1][src0],
        fluxes[0][tgt0],
        fluxes[1][tgt0],
    ])  # [5, E]: fdo, fs0, fs1, ft0, ft1   (no self loops)

    perm = np.argsort(tgt0, kind="stable")
    src_s = src0[perm]
    tgt_s = tgt0[perm]
    per_edge = per_edge_all[:, perm]

    tile_starts = np.searchsorted(tgt_s, np.arange(0, n + 1, cfg.tile))
    is_a = src_s < cfg.split

    ct, ntc, ctn = cfg.ct, cfg.ntc, cfg.ctn
    T = cfg.tile

    shared = {
        "xpa": np.ascontiguousarray(xi[: cfg.na]),
        "xpb": np.ascontiguousarray(xi[cfg.na:]),
        # wide iota, t-major: col t*ct + c = t
        "iotaw": np.tile(np.repeat(np.arange(T, dtype=np.float32), ct),
                         (128, 1)).astype(BF16),
        "ident": np.eye(128, dtype=np.float32).astype(BF16),
    }

    names = ["fdo", "fs0", "fs1", "ft0", "ft1"]
    npc = cfg.nodes_per_core
    cores = []
    for core in range(cfg.n_cores):
        tl_all = np.full((128, ctn), -1.0, np.float32)
        pe_all = np.zeros((5, 128, ctn), np.float32)
        idx16 = np.zeros((128, ctn * 8), np.int16)
        degown = np.ones((128, ntc), np.float32)
        for tt in range(ntc):
            t = core * ntc + tt
            t0 = t * cfg.tile
            s, e = tile_starts[t], tile_starts[t + 1]
            sel_a = np.nonzero(is_a[s:e])[0] + s
            sel_b = np.nonzero(~is_a[s:e])[0] + s
            nA, nB = len(sel_a), len(sel_b)
            assert nA <= cfg.capa * CHUNK, (t, nA, cfg.capa * CHUNK)
            assert nB <= cfg.capb * CHUNK, (t, nB, cfg.capb * CHUNK)

            ids = np.zeros(ct * CHUNK, np.int64)
            tl = np.full(ct * CHUNK, -1.0, np.float32)
            pe = np.zeros((5, ct * CHUNK), np.float32)
            off = cfg.capa * CHUNK
            ids[:nA] = src_s[sel_a]
            ids[off:off + nB] = src_s[sel_b] - cfg.na
            tl[:nA] = tgt_s[sel_a] - t0
            tl[off:off + nB] = tgt_s[sel_b] - t0
            pe[:, :nA] = per_edge[:, sel_a]
            pe[:, off:off + nB] = per_edge[:, sel_b]

            # chunk-transposed layout: element (p, c) = edge[c*128+p]
            cols = slice(tt * ct, (tt + 1) * ct)
            tl_all[:, cols] = tl.reshape(ct, CHUNK).T
            for j in range(5):
                pe_all[j][:, cols] = pe[j].reshape(ct, CHUNK).T

            islice = idx16[:, tt * ct * 8:(tt + 1) * ct * 8]
            islice[:, : cfg.capa * 8] = _wrap16(ids[:off])
            islice[:, cfg.capa * 8:] = _wrap16(ids[off:])

            degown[:cfg.tile, tt] = deg[t0:t0 + cfg.tile]
        d = {"tl": tl_all.astype(BF16), "idx16": idx16, "degown": degown,
             "yloop": np.ascontiguousarray(xi[core * npc:(core + 1) * npc])}
        for j, nm in enumerate(names):
            d[nm] = np.ascontiguousarray(pe_all[j])
        cores.append(d)
    return shared, cores


# -------------------- device program --------------------

def build_nc(cfg: Cfg):
    import concourse.bass as bass
    import concourse.tile as tile
    from concourse import bacc, mybir

    dt = mybir.dt
    act = mybir.ActivationFunctionType
    alu = mybir.AluOpType

    ct, capa, capb = cfg.ct, cfg.capa, cfg.capb
    ntc, T, ctn = cfg.ntc, cfg.tile, cfg.ctn

    nc = bacc.Bacc("TRN2", target_bir_lowering=False, debug=False,
                   num_swdge_queues=4)

    xpa = nc.dram_tensor("xpa", [cfg.na, 2 * C], dt.bfloat16, kind="ExternalInput")
    xpb = nc.dram_tensor("xpb", [cfg.nb, 2 * C], dt.bfloat16, kind="ExternalInput")
    yl_d = nc.dram_tensor("yloop", [ntc * T, 2 * C], dt.bfloat16, kind="ExternalInput")
    tl_d = nc.dram_tensor("tl", [128, ctn], dt.bfloat16, kind="ExternalInput")
    fdo_d = nc.dram_tensor("fdo", [128, ctn], dt.float32, kind="ExternalInput")
    fs0_d = nc.dram_tensor("fs0", [128, ctn], dt.float32, kind="ExternalInput")
    fs1_d = nc.dram_tensor("fs1", [128, ctn], dt.float32, kind="ExternalInput")
    ft0_d = nc.dram_tensor("ft0", [128, ctn], dt.float32, kind="ExternalInput")
    ft1_d = nc.dram_tensor("ft1", [128, ctn], dt.float32, kind="ExternalInput")
    idx16_d = nc.dram_tensor("idx16", [128, ctn * 8], dt.int16, kind="ExternalInput")
    degown_d = nc.dram_tensor("degown", [128, ntc], dt.float32, kind="ExternalInput")
    iotaw_d = nc.dram_tensor("iotaw", [128, T * ct], dt.bfloat16, kind="ExternalInput")
    ident_d = nc.dram_tensor("ident", [128, 128], dt.bfloat16, kind="ExternalInput")
    wct_d = nc.dram_tensor("wct2", [128, C], dt.bfloat16, kind="ExternalInput")
    wdt_d = nc.dram_tensor("wdt2", [128, C], dt.bfloat16, kind="ExternalInput")
    bias_d = nc.dram_tensor("biasr", [128, C], dt.float32, kind="ExternalInput")
    out0 = nc.dram_tensor("out0", [ntc * T, C], dt.float32, kind="ExternalOutput")
    out1 = nc.dram_tensor("out1", [ntc * T, C], dt.float32, kind="ExternalOutput")
    outs = [out0, out1]

    with tile.TileContext(nc) as tc:
        with (
            tc.tile_pool(name="const", bufs=1) as constp,
            tc.tile_pool(name="res", bufs=1) as resp,
        ):
            iotaw_sb = constp.tile([128, T * ct], dt.bfloat16)
            nc.sync.dma_start(iotaw_sb[:], iotaw_d[:, :])
            ident_sb = constp.tile([128, 128], dt.bfloat16)
            nc.sync.dma_start(ident_sb[:], ident_d[:, :])
            bias_sb = constp.tile([128, C], dt.float32)
            nc.sync.dma_start(bias_sb[:], bias_d[:, :])
            wct_sb = constp.tile([128, C], dt.bfloat16)
            nc.sync.dma_start(wct_sb[:], wct_d[:, :])
            wdt_sb = constp.tile([128, C], dt.bfloat16)
            nc.sync.dma_start(wdt_sb[:], wdt_d[:, :])
            # wdc = Wd - Wc (projection uses T*Wc + V*(Wd-Wc))
            wdc_sb = constp.tile([128, C], dt.bfloat16)
            nc.vector.tensor_tensor(wdc_sb[:], wdt_sb[:], wct_sb[:],
                                    alu.subtract)

            # resident per-core data
            tl_sb = resp.tile([128, ctn], dt.bfloat16)
            nc.sync.dma_start(tl_sb[:], tl_d[:, :])
            idx_sb = resp.tile([128, ctn * 8], dt.int16)
            nc.sync.dma_start(idx_sb[:], idx16_d[:, :])
            # f for both batches, interleaved [128, ctn, 2] bf16
            fcat_sb = resp.tile([128, ctn * 2], dt.bfloat16)
            fcat3 = fcat_sb[:].rearrange("p (n b) -> p n b", b=2)

            # dis for own target nodes (fp32, output scale)
            disown_sb = resp.tile([128, ntc], dt.float32)
            nc.sync.dma_start(disown_sb[:], degown_d[:, :])
            nc.vector.reciprocal(disown_sb[:], disown_sb[:])
            nc.scalar.activation(disown_sb[:], disown_sb[:], act.Sqrt)

            # ---- prepass: f0/f1 from flux/fdo ----
            with tc.tile_pool(name="pp", bufs=1) as ppp:
                fdo_sb = ppp.tile([128, ctn], dt.float32)
                nc.sync.dma_start(fdo_sb[:], fdo_d[:, :])
                c1 = ppp.tile([128, ctn], dt.float32)
                nc.vector.tensor_scalar(
                    c1[:], fdo_sb[:], 2.0, -1.0, alu.mult, alu.add)
                c0 = ppp.tile([128, ctn], dt.float32)
                nc.vector.tensor_scalar(
                    c0[:], fdo_sb[:], -1.0, 1.0, alu.mult, alu.add)
                for b, (fsd, ftd) in enumerate(((fs0_d, ft0_d), (fs1_d, ft1_d))):
                    fs_sb = ppp.tile([128, ctn], dt.float32, tag="fs")
                    nc.sync.dma_start(fs_sb[:], fsd[:, :])
                    ft_sb = ppp.tile([128, ctn], dt.float32, tag="ft")
                    nc.sync.dma_start(ft_sb[:], ftd[:, :])
                    nc.vector.tensor_mul(fs_sb[:], fs_sb[:], ft_sb[:])
                    nc.scalar.activation(
                        ft_sb[:], fs_sb[:], act.Sigmoid, scale=2.0)
                    nc.vector.tensor_mul(fs_sb[:], ft_sb[:], c1[:])
                    nc.vector.tensor_add(fcat3[:, :, b], fs_sb[:], c0[:])


            # ---- main loop over node tiles ----
            with (
                tc.tile_pool(name="xg", bufs=3) as xgp,
                tc.tile_pool(name="yl", bufs=2) as ylp,
                tc.tile_pool(name="oe", bufs=2) as oep,
                tc.tile_pool(name="tvs", bufs=2) as tvsp,
                tc.tile_pool(name="trs", bufs=2) as trsp,
                tc.tile_pool(name="outp", bufs=2) as outsp,
                tc.tile_pool(name="ps_tv", bufs=2, space="PSUM") as pstv,
                tc.tile_pool(name="ps_tr", bufs=2, space="PSUM") as pstr,
                tc.tile_pool(name="ps_o", bufs=2, space="PSUM") as pso,
            ):
                for tt in range(ntc):
                    ib = tt * ct * 8
                    # gathered rows (first half) | f-scaled rows (second half)
                    xg = xgp.tile([128, 2 * ct * 2 * C], dt.bfloat16, tag="xg")
                    xg3 = xg[:].rearrange("p (c r) -> p c r", r=2 * C)
                    ca1 = capa // 2
                    cb1 = capb // 2
                    gathers = [
                        (0, ca1, xpa, 0),
                        (ca1, capa, xpa, 1),
                        (capa, capa + cb1, xpb, 2),
                        (capa + cb1, ct, xpb, 3),
                    ]
                    for (c0_, c1_, tab, q) in gathers:
                        nch = c1_ - c0_
                        nc.gpsimd.dma_gather(
                            xg3[:, c0_:c1_],
                            tab[:, :],
                            idx_sb[:, ib + c0_ * 8: ib + c1_ * 8],
                            nch * CHUNK, nch * CHUNK, 2 * C,
                            single_packet=False, queue_num=q,
                        )

                    # self-loop rows for this tile (contiguous stream)
                    yl_sb = ylp.tile([128, 2 * C], dt.bfloat16, tag="yl")
                    nc.sync.dma_start(
                        yl_sb[:T, :], yl_d[tt * T:(tt + 1) * T, :])

                    # one-hot, t-major (col = t*ct + c); table is
                    # dis_src-prescaled so values are pure 0/1
                    o_all = oep.tile([128, T * ct], dt.bfloat16, tag="oe")
                    o3 = o_all[:].rearrange("p (t c) -> p t c", c=ct)
                    tl_cols = tl_sb[:, tt * ct:(tt + 1) * ct].unsqueeze(1)
                    nc.vector.tensor_tensor(
                        o3,
                        tl_cols.to_broadcast([128, T, ct]),
                        iotaw_sb[:].rearrange("p (t c) -> p t c", c=ct),
                        alu.is_equal,
                    )

                    # second half = f * first half (per batch, pair-interleaved)
                    xh4 = xg[:, 0:ct * 2 * C].rearrange(
                        "p (c f b) -> p c f b", f=C, b=2)
                    fxh4 = xg[:, ct * 2 * C:2 * ct * 2 * C].rearrange(
                        "p (c f b) -> p c f b", f=C, b=2)
                    fcols = fcat3[:, tt * ct:(tt + 1) * ct, :].unsqueeze(2)
                    nc.vector.tensor_tensor(
                        fxh4, xh4, fcols.to_broadcast([128, ct, C, 2]),
                        alu.mult)

                    # [T | V] accumulation: one matmul per chunk, 256-wide rhs
                    tv_ps = pstv.tile([T, 4 * C], dt.float32, tag="tv")
                    xgv = xg[:].rearrange("p (h w) -> p h w", h=2)
                    for c in range(ct):
                        nc.tensor.matmul(
                            out=tv_ps[:],
                            lhsT=o3[:, :, c],
                            rhs=xgv[:, :, c * 2 * C:(c + 1) * 2 * C],
                            start=(c == 0), stop=(c == ct - 1),
                        )

                    # epilogue: deinterleaving bf16 copy (pair-interleaved ->
                    # batch-major) + self-loop add, transposes to [ch, t],
                    # project
                    tv_sb = tvsp.tile([T, 4 * C], dt.bfloat16, tag="tv_sb")
                    nc.scalar.activation(
                        tv_sb[:].rearrange("p (g b f) -> p g b f",
                                           g=2, b=2, f=C),
                        tv_ps[:].rearrange("p (g f b) -> p g b f",
                                           g=2, f=C, b=2),
                        act.Copy)
                    nc.vector.tensor_add(
                        tv_sb[:, 0:2 * C].rearrange("p (b f) -> p b f", b=2),
                        tv_sb[:, 0:2 * C].rearrange("p (b f) -> p b f", b=2),
                        yl_sb[:T, :].rearrange("p (f b) -> p b f", b=2))
                    tr_ps = pstr.tile([128, 256], dt.bfloat16, tag="tr")
                    nc.tensor.transpose(
                        tr_ps[:, 0:T], tv_sb[:, 0:2 * C], ident_sb[:T, :T])
                    nc.tensor.transpose(
                        tr_ps[:, 128:128 + T], tv_sb[:, 2 * C:4 * C],
                        ident_sb[:T, :T])
                    tr_sb = trsp.tile([128, 256], dt.bfloat16, tag="tr_sb")
                    nc.vector.tensor_copy(
                        out=tr_sb[:, 0:T], in_=tr_ps[:, 0:T])
                    nc.scalar.activation(
                        tr_sb[:, 128:128 + T], tr_ps[:, 128:128 + T], act.Copy)

                    for bi in range(2):
                        rows = slice(64 * bi, 64 * bi + 64)
                        op_ps = pso.tile([T, C], dt.float32, tag=f"op{bi}")
                        nc.tensor.matmul(
                            out=op_ps[:], lhsT=tr_sb[rows, 0:T],
                            rhs=wct_sb[rows, :],
                            start=True, stop=False,
                        )
                        nc.tensor.matmul(
                            out=op_ps[:], lhsT=tr_sb[rows, 128:128 + T],
                            rhs=wdc_sb[rows, :],
                            start=False, stop=True,
                        )
                        o_sb = outsp.tile([128, C], dt.float32, tag=f"os{bi}")
                        nc.vector.tensor_scalar(
                            o_sb[:T, :], op_ps[:], disown_sb[:T, tt:tt + 1],
                            None, alu.mult)
                        nc.vector.tensor_add(
                            o_sb[:T, :], o_sb[:T, :], bias_sb[:T, :])
                        nc.sync.dma_start(
                            outs[bi][tt * T:(tt + 1) * T, :], o_sb[:T, :])

    nc.compile()
    return nc


def _shared_weights(W_conc, W_disc, bias):
    wct2 = np.zeros((128, C), np.float32)
    wdt2 = np.zeros((128, C), np.float32)
    wct2[:64] = np.asarray(W_conc, np.float32).T  # WcT[i, o] = Wc[o, i]
    wct2[64:] = wct2[:64]
    wdt2[:64] = np.asarray(W_disc, np.float32).T
    wdt2[64:] = wdt2[:64]
    biasr = np.tile(np.asarray(bias, np.float32)[None, :], (128, 1))
    return wct2.astype(BF16), wdt2.astype(BF16), biasr


_NC_CACHE = {}


def _caps_needed(edge_index, n, n_cores, tile, split):
    """Max per-tile chunk counts for the A/B table split (no self loops)."""
    src0 = np.asarray(edge_index[0]).astype(np.int64)
    tgt0 = np.asarray(edge_index[1]).astype(np.int64)
    order = np.argsort(tgt0, kind="stable")
    tgt_s, src_s = tgt0[order], src0[order]
    starts = np.searchsorted(tgt_s, np.arange(0, n + 1, tile))
    na = np.add.reduceat((src_s < split).astype(np.int64), starts[:-1])
    tot = np.diff(starts)
    maxa = int(na.max())
    maxb = int((tot - na).max())
    return -(-maxa // CHUNK), -(-maxb // CHUNK)


def _make_in_maps(x, edge_index, f_disc_orig, fluxes, W_conc, W_disc, bias,
                  cfg):
    shared, cores = prep(x, edge_index, f_disc_orig, fluxes, cfg)
    wct2, wdt2, biasr = _shared_weights(W_conc, W_disc, bias)
    in_maps = []
    for core in range(cfg.n_cores):
        m = dict(shared)
        m.update(cores[core])
        m["wct2"] = wct2
        m["wdt2"] = wdt2
        m["biasr"] = biasr
        in_maps.append(m)
    return in_maps


def _run(inputs, trace=False):
    from concourse.bass_utils import run_bass_kernel_spmd

    x = np.asarray(inputs["x"], np.float32)
    n = x.shape[1]
    capa, capb = _caps_needed(inputs["edge_index"], n, N_CORES, TILE, SPLIT)
    cfg = Cfg(n_nodes=n, n_cores=N_CORES, tile=TILE, split=SPLIT,
              capa=max(capa, 2), capb=max(capb, 2))
    in_maps = _make_in_maps(
        x, inputs["edge_index"], inputs["f_disc_orig"], inputs["fluxes"],
        inputs["W_conc"], inputs["W_disc"], inputs["bias"], cfg)

    if cfg not in _NC_CACHE:
        _NC_CACHE[cfg] = build_nc(cfg)
    nc = _NC_CACHE[cfg]

    res = run_bass_kernel_spmd(nc, in_maps, list(range(cfg.n_cores)),
                               trace=trace)
    out = np.zeros((BATCH, n, C), np.float32)
    npc = cfg.nodes_per_core
    for core in range(cfg.n_cores):
        out[0, core * npc:(core + 1) * npc] = res.results[core]["out0"]
        out[1, core * npc:(core + 1) * npc] = res.results[core]["out1"]
    return out, res


def kernel(x, edge_index, f_disc_orig, fluxes, W_conc, W_disc, bias):
    out, _ = _run(dict(x=x, edge_index=edge_index, f_disc_orig=f_disc_orig,
                       fluxes=fluxes, W_conc=W_conc, W_disc=W_disc, bias=bias))
    return out


def profile_run(inputs):
    out, res = _run(inputs, trace=True)
    return res.exec_time_ns


# revision 22
# speedup vs baseline: 1.4434x; 1.0757x over previous
"""Trainium2 Bass kernel for nn_ReaReaConv (GCN-style message passing with
dynamic edge gating).

Math (per batch b):
    deg[n]   = in-degree(n) + 1 (self loop);  dis = rsqrt(deg)
    f_e      = keep*fdo + (1-keep)*(1-fdo), keep = sigmoid(2*flux[src]*flux[tgt])
    out[t]   = dis_t * ( T[t] @ Wc^T + V[t] @ (Wd-Wc)^T ) + bias
    T[t]     = sum_{e->t} dis_src * x[src_e] + dis_t * x[t]   (self loop f=0)
    V[t]     = sum_{e->t} dis_src * f_e * x[src_e]

Sharding: each of the 8 cores owns N/8 target nodes (tiles of 125). Host sorts
edges by target tile, prescales the gather table by dis_src and casts to bf16
(index/layout/elementwise-prescale only; per-edge math runs on device).

v4 design notes:
 - Gather table rows are batch-PAIR-interleaved ([x0[c],x1[c]] pairs) so the
   on-device f-scale multiply qualifies for the DVE 2x 16-bit mode (the f
   broadcast lands mid-dim, innermost is the contiguous batch pair).
 - One-hot is built t-major (col = t*ct + c) against a host-built wide iota so
   every operand of the is_equal has innermost stride 1 -> DVE 2x mode.
 - Self-loop contributions are streamed contiguously (no gather descriptors)
   and added to T as a [125, 128] DVE add.
 - Per 128-edge chunk: ONE matmul, lhsT = one-hot [128, 125], rhs = [y | f*y]
   [128, 256] accumulating PSUM [125, 256] = [T | V]. Epilogue PE-transposes
   (deinterleaving batch pairs for free) and projects with Wc / (Wd-Wc).
"""

from dataclasses import dataclass

import ml_dtypes
import numpy as np

BF16 = ml_dtypes.bfloat16

# -------------------- problem constants --------------------
N_NODES = 50000
N_EDGES = 1600000
BATCH = 2
C = 64
N_CORES = 8
TILE = 125           # target nodes per tile (one-hot width)
CHUNK = 128          # edges per matmul chunk (PE contraction)
SPLIT = 32768        # gather-table split (int16 signed index limit)


@dataclass(frozen=True)
class Cfg:
    n_nodes: int
    n_cores: int
    tile: int
    split: int
    capa: int  # chunks per tile from table A (src < split)
    capb: int  # chunks per tile from table B

    @property
    def nodes_per_core(self):
        return self.n_nodes // self.n_cores

    @property
    def ntc(self):  # tiles per core
        return self.nodes_per_core // self.tile

    @property
    def ct(self):
        return self.capa + self.capb

    @property
    def ctn(self):
        return self.ntc * self.ct

    @property
    def na(self):
        return min(self.split, self.n_nodes)

    @property
    def nb(self):
        return self.n_nodes - self.na


# -------------------- host prep (indices / layout / dtype cast) ------------

def _wrap16(idx_flat):
    """dma_gather index layout: [128, n/16] int16, idx[p, s] = flat[s*16+p],
    replicated across the 8 gpsimd cores (partition blocks of 16)."""
    n = len(idx_flat)
    assert n % 16 == 0
    w = np.asarray(idx_flat, np.int16).reshape(n // 16, 16).T  # [16, n/16]
    return np.tile(w, (8, 1))  # [128, n/16]


def prep(x, edge_index, f_disc_orig, fluxes, cfg: Cfg):
    """Returns (shared dict, list of per-core dicts). Index/layout/cast and
    elementwise dis prescale only — per-edge math runs on device."""
    n = cfg.n_nodes
    src0 = np.asarray(edge_index[0]).astype(np.int64)
    tgt0 = np.asarray(edge_index[1]).astype(np.int64)
    x = np.asarray(x, np.float32)
    fdo_in = np.asarray(f_disc_orig, np.float32)
    fluxes = np.asarray(fluxes, np.float32)

    deg = (np.bincount(tgt0, minlength=n) + 1).astype(np.float32)  # int-valued
    dis = 1.0 / np.sqrt(deg)

    # pair-interleaved, dis-prescaled bf16 gather table:
    # row n = [x0[n,0], x1[n,0], x0[n,1], x1[n,1], ...] * dis_n
    xi = np.empty((n, 2 * C), np.float32)
    xi[:, 0::2] = x[0] * dis[:, None]
    xi[:, 1::2] = x[1] * dis[:, None]
    xi = xi.astype(BF16)

    per_edge_all = np.stack([
        fdo_in,
        fluxes[0][src0],
        fluxes[1][src0],
        fluxes[0][tgt0],
        fluxes[1][tgt0],
    ])  # [5, E]: fdo, fs0, fs1, ft0, ft1   (no self loops)

    perm = np.argsort(tgt0, kind="stable")
    src_s = src0[perm]
    tgt_s = tgt0[perm]
    per_edge = per_edge_all[:, perm]

    tile_starts = np.searchsorted(tgt_s, np.arange(0, n + 1, cfg.tile))
    is_a = src_s < cfg.split

    ct, ntc, ctn = cfg.ct, cfg.ntc, cfg.ctn
    T = cfg.tile

    shared = {
        "xpa": np.ascontiguousarray(xi[: cfg.na]),
        "xpb": np.ascontiguousarray(xi[cfg.na:]),
        # wide iota, t-major: col t*ct + c = t
        "iotaw": np.tile(np.repeat(np.arange(T, dtype=np.float32), ct),
                         (128, 1)).astype(BF16),
        "ident": np.eye(128, dtype=np.float32).astype(BF16),
    }

    names = ["fdo", "fs0", "fs1", "ft0", "ft1"]
    npc = cfg.nodes_per_core
    cores = []
    for core in range(cfg.n_cores):
        tl_all = np.full((128, ctn), -1.0, np.float32)
        pe_all = np.zeros((5, 128, ctn), np.float32)
        idx16 = np.zeros((128, ctn * 8), np.int16)
        degown = np.ones((128, ntc), np.float32)
        for tt in range(ntc):
            t = core * ntc + tt
            t0 = t * cfg.tile
            s, e = tile_starts[t], tile_starts[t + 1]
            sel_a = np.nonzero(is_a[s:e])[0] + s
            sel_b = np.nonzero(~is_a[s:e])[0] + s
            nA, nB = len(sel_a), len(sel_b)
            assert nA <= cfg.capa * CHUNK, (t, nA, cfg.capa * CHUNK)
            assert nB <= cfg.capb * CHUNK, (t, nB, cfg.capb * CHUNK)

            ids = np.zeros(ct * CHUNK, np.int64)
            tl = np.full(ct * CHUNK, -1.0, np.float32)
            pe = np.zeros((5, ct * CHUNK), np.float32)
            off = cfg.capa * CHUNK
            ids[:nA] = src_s[sel_a]
            ids[off:off + nB] = src_s[sel_b] - cfg.na
            tl[:nA] = tgt_s[sel_a] - t0
            tl[off:off + nB] = tgt_s[sel_b] - t0
            pe[:, :nA] = per_edge[:, sel_a]
            pe[:, off:off + nB] = per_edge[:, sel_b]

            # chunk-transposed layout: element (p, c) = edge[c*128+p]
            cols = slice(tt * ct, (tt + 1) * ct)
            tl_all[:, cols] = tl.reshape(ct, CHUNK).T
            for j in range(5):
                pe_all[j][:, cols] = pe[j].reshape(ct, CHUNK).T

            islice = idx16[:, tt * ct * 8:(tt + 1) * ct * 8]
            islice[:, : cfg.capa * 8] = _wrap16(ids[:off])
            islice[:, cfg.capa * 8:] = _wrap16(ids[off:])

            degown[:cfg.tile, tt] = deg[t0:t0 + cfg.tile]
        d = {"tl": tl_all.astype(BF16), "idx16": idx16, "degown": degown,
             "yloop": np.ascontiguousarray(xi[core * npc:(core + 1) * npc])}
        for j, nm in enumerate(names):
            d[nm] = np.ascontiguousarray(pe_all[j])
        cores.append(d)
    return shared, cores


# -------------------- device program --------------------

def build_nc(cfg: Cfg):
    import concourse.bass as bass
    import concourse.tile as tile
    from concourse import bacc, mybir

    dt = mybir.dt
    act = mybir.ActivationFunctionType
    alu = mybir.AluOpType

    ct, capa, capb = cfg.ct, cfg.capa, cfg.capb
    ntc, T, ctn = cfg.ntc, cfg.tile, cfg.ctn

    nc = bacc.Bacc("TRN2", target_bir_lowering=False, debug=False,
                   num_swdge_queues=4)

    xpa = nc.dram_tensor("xpa", [cfg.na, 2 * C], dt.bfloat16, kind="ExternalInput")
    xpb = nc.dram_tensor("xpb", [cfg.nb, 2 * C], dt.bfloat16, kind="ExternalInput")
    yl_d = nc.dram_tensor("yloop", [ntc * T, 2 * C], dt.bfloat16, kind="ExternalInput")
    tl_d = nc.dram_tensor("tl", [128, ctn], dt.bfloat16, kind="ExternalInput")
    fdo_d = nc.dram_tensor("fdo", [128, ctn], dt.float32, kind="ExternalInput")
    fs0_d = nc.dram_tensor("fs0", [128, ctn], dt.float32, kind="ExternalInput")
    fs1_d = nc.dram_tensor("fs1", [128, ctn], dt.float32, kind="ExternalInput")
    ft0_d = nc.dram_tensor("ft0", [128, ctn], dt.float32, kind="ExternalInput")
    ft1_d = nc.dram_tensor("ft1", [128, ctn], dt.float32, kind="ExternalInput")
    idx16_d = nc.dram_tensor("idx16", [128, ctn * 8], dt.int16, kind="ExternalInput")
    degown_d = nc.dram_tensor("degown", [128, ntc], dt.float32, kind="ExternalInput")
    iotaw_d = nc.dram_tensor("iotaw", [128, T * ct], dt.bfloat16, kind="ExternalInput")
    ident_d = nc.dram_tensor("ident", [128, 128], dt.bfloat16, kind="ExternalInput")
    wct_d = nc.dram_tensor("wct2", [128, C], dt.bfloat16, kind="ExternalInput")
    wdt_d = nc.dram_tensor("wdt2", [128, C], dt.bfloat16, kind="ExternalInput")
    bias_d = nc.dram_tensor("biasr", [128, C], dt.float32, kind="ExternalInput")
    out0 = nc.dram_tensor("out0", [ntc * T, C], dt.float32, kind="ExternalOutput")
    out1 = nc.dram_tensor("out1", [ntc * T, C], dt.float32, kind="ExternalOutput")
    outs = [out0, out1]

    with tile.TileContext(nc) as tc:
        with (
            tc.tile_pool(name="const", bufs=1) as constp,
            tc.tile_pool(name="res", bufs=1) as resp,
        ):
            iotaw_sb = constp.tile([128, T * ct], dt.bfloat16)
            nc.sync.dma_start(iotaw_sb[:], iotaw_d[:, :])
            ident_sb = constp.tile([128, 128], dt.bfloat16)
            nc.sync.dma_start(ident_sb[:], ident_d[:, :])
            bias_sb = constp.tile([128, C], dt.float32)
            nc.sync.dma_start(bias_sb[:], bias_d[:, :])
            wct_sb = constp.tile([128, C], dt.bfloat16)
            nc.sync.dma_start(wct_sb[:], wct_d[:, :])
            wdt_sb = constp.tile([128, C], dt.bfloat16)
            nc.sync.dma_start(wdt_sb[:], wdt_d[:, :])
            # wdc = Wd - Wc (projection uses T*Wc + V*(Wd-Wc))
            wdc_sb = constp.tile([128, C], dt.bfloat16)
            nc.vector.tensor_tensor(wdc_sb[:], wdt_sb[:], wct_sb[:],
                                    alu.subtract)

            # resident per-core data
            tl_sb = resp.tile([128, ctn], dt.bfloat16)
            nc.sync.dma_start(tl_sb[:], tl_d[:, :])
            idx_sb = resp.tile([128, ctn * 8], dt.int16)
            nc.sync.dma_start(idx_sb[:], idx16_d[:, :])
            # f for both batches, interleaved [128, ctn, 2] bf16
            fcat_sb = resp.tile([128, ctn * 2], dt.bfloat16)
            fcat3 = fcat_sb[:].rearrange("p (n b) -> p n b", b=2)

            # dis for own target nodes (fp32, output scale)
            disown_sb = resp.tile([128, ntc], dt.float32)
            nc.sync.dma_start(disown_sb[:], degown_d[:, :])
            nc.vector.reciprocal(disown_sb[:], disown_sb[:])
            nc.scalar.activation(disown_sb[:], disown_sb[:], act.Sqrt)

            # ---- prepass: f0/f1 from flux/fdo ----
            with tc.tile_pool(name="pp", bufs=1) as ppp:
                fdo_sb = ppp.tile([128, ctn], dt.float32)
                nc.sync.dma_start(fdo_sb[:], fdo_d[:, :])
                c1 = ppp.tile([128, ctn], dt.float32)
                nc.vector.tensor_scalar(
                    c1[:], fdo_sb[:], 2.0, -1.0, alu.mult, alu.add)
                c0 = ppp.tile([128, ctn], dt.float32)
                nc.vector.tensor_scalar(
                    c0[:], fdo_sb[:], -1.0, 1.0, alu.mult, alu.add)
                for b, (fsd, ftd) in enumerate(((fs0_d, ft0_d), (fs1_d, ft1_d))):
                    fs_sb = ppp.tile([128, ctn], dt.float32, tag="fs")
                    nc.sync.dma_start(fs_sb[:], fsd[:, :])
                    ft_sb = ppp.tile([128, ctn], dt.float32, tag="ft")
                    nc.sync.dma_start(ft_sb[:], ftd[:, :])
                    nc.vector.tensor_mul(fs_sb[:], fs_sb[:], ft_sb[:])
                    nc.scalar.activation(
                        ft_sb[:], fs_sb[:], act.Sigmoid, scale=2.0)
                    nc.vector.tensor_mul(fs_sb[:], ft_sb[:], c1[:])
                    nc.vector.tensor_add(fcat3[:, :, b], fs_sb[:], c0[:])


            # ---- main loop over node tiles ----
            with (
                tc.tile_pool(name="xg", bufs=3) as xgp,
                tc.tile_pool(name="yl", bufs=2) as ylp,
                tc.tile_pool(name="oe", bufs=2) as oep,
                tc.tile_pool(name="tvs", bufs=2) as tvsp,
                tc.tile_pool(name="trs", bufs=2) as trsp,
                tc.tile_pool(name="outp", bufs=2) as outsp,
                tc.tile_pool(name="ps_tv", bufs=2, space="PSUM") as pstv,
                tc.tile_pool(name="ps_tr", bufs=2, space="PSUM") as pstr,
                tc.tile_pool(name="ps_o", bufs=2, space="PSUM") as pso,
            ):
                # balanced queue split: ~ct/4 chunks per SWDGE queue so the
                # per-tile gather latency tracks ct/4, not max(capa/2, capb)
                gathers = []
                pos = 0
                for q in range(4):
                    take = (ct + 3 - q) // 4
                    while take > 0:
                        if pos < capa:
                            nn = min(take, capa - pos)
                            gathers.append((pos, pos + nn, True, q))
                        else:
                            nn = min(take, ct - pos)
                            gathers.append((pos, pos + nn, False, q))
                        pos += nn
                        take -= nn

                for tt in range(ntc):
                    ib = tt * ct * 8
                    # gathered rows (first half) | f-scaled rows (second half)
                    xg = xgp.tile([128, 2 * ct * 2 * C], dt.bfloat16, tag="xg")
                    xg3 = xg[:].rearrange("p (c r) -> p c r", r=2 * C)
                    for (c0_, c1_, is_a_, q) in gathers:
                        nch = c1_ - c0_
                        nc.gpsimd.dma_gather(
                            xg3[:, c0_:c1_],
                            (xpa if is_a_ else xpb)[:, :],
                            idx_sb[:, ib + c0_ * 8: ib + c1_ * 8],
                            nch * CHUNK, nch * CHUNK, 2 * C,
                            single_packet=False, queue_num=q,
                        )

                    # self-loop rows for this tile (contiguous stream)
                    yl_sb = ylp.tile([128, 2 * C], dt.bfloat16, tag="yl")
                    nc.sync.dma_start(
                        yl_sb[:T, :], yl_d[tt * T:(tt + 1) * T, :])

                    # one-hot, t-major (col = t*ct + c); table is
                    # dis_src-prescaled so values are pure 0/1
                    o_all = oep.tile([128, T * ct], dt.bfloat16, tag="oe")
                    o3 = o_all[:].rearrange("p (t c) -> p t c", c=ct)
                    tl_cols = tl_sb[:, tt * ct:(tt + 1) * ct].unsqueeze(1)
                    nc.vector.tensor_tensor(
                        o3,
                        tl_cols.to_broadcast([128, T, ct]),
                        iotaw_sb[:].rearrange("p (t c) -> p t c", c=ct),
                        alu.is_equal,
                    )

                    # second half = f * first half (per batch, pair-interleaved)
                    xh4 = xg[:, 0:ct * 2 * C].rearrange(
                        "p (c f b) -> p c f b", f=C, b=2)
                    fxh4 = xg[:, ct * 2 * C:2 * ct * 2 * C].rearrange(
                        "p (c f b) -> p c f b", f=C, b=2)
                    fcols = fcat3[:, tt * ct:(tt + 1) * ct, :].unsqueeze(2)
                    nc.vector.tensor_tensor(
                        fxh4, xh4, fcols.to_broadcast([128, ct, C, 2]),
                        alu.mult)

                    # [T | V] accumulation: one matmul per chunk, 256-wide rhs
                    tv_ps = pstv.tile([T, 4 * C], dt.float32, tag="tv")
                    xgv = xg[:].rearrange("p (h w) -> p h w", h=2)
                    for c in range(ct):
                        nc.tensor.matmul(
                            out=tv_ps[:],
                            lhsT=o3[:, :, c],
                            rhs=xgv[:, :, c * 2 * C:(c + 1) * 2 * C],
                            start=(c == 0), stop=(c == ct - 1),
                        )

                    # epilogue: deinterleaving bf16 copy (pair-interleaved ->
                    # batch-major) + self-loop add, transposes to [ch, t],
                    # project
                    tv_sb = tvsp.tile([T, 4 * C], dt.bfloat16, tag="tv_sb")
                    nc.scalar.activation(
                        tv_sb[:].rearrange("p (g b f) -> p g b f",
                                           g=2, b=2, f=C),
                        tv_ps[:].rearrange("p (g f b) -> p g b f",
                                           g=2, f=C, b=2),
                        act.Copy)
                    nc.vector.tensor_add(
                        tv_sb[:, 0:2 * C].rearrange("p (b f) -> p b f", b=2),
                        tv_sb[:, 0:2 * C].rearrange("p (b f) -> p b f", b=2),
                        yl_sb[:T, :].rearrange("p (f b) -> p b f", b=2))
                    tr_ps = pstr.tile([128, 256], dt.bfloat16, tag="tr")
                    nc.tensor.transpose(
                        tr_ps[:, 0:T], tv_sb[:, 0:2 * C], ident_sb[:T, :T])
                    nc.tensor.transpose(
                        tr_ps[:, 128:128 + T], tv_sb[:, 2 * C:4 * C],
                        ident_sb[:T, :T])
                    tr_sb = trsp.tile([128, 256], dt.bfloat16, tag="tr_sb")
                    nc.vector.tensor_copy(
                        out=tr_sb[:, 0:T], in_=tr_ps[:, 0:T])
                    nc.scalar.activation(
                        tr_sb[:, 128:128 + T], tr_ps[:, 128:128 + T], act.Copy)

                    for bi in range(2):
                        rows = slice(64 * bi, 64 * bi + 64)
                        op_ps = pso.tile([T, C], dt.float32, tag=f"op{bi}")
                        nc.tensor.matmul(
                            out=op_ps[:], lhsT=tr_sb[rows, 0:T],
                            rhs=wct_sb[rows, :],
                            start=True, stop=False,
                        )
                        nc.tensor.matmul(
                            out=op_ps[:], lhsT=tr_sb[rows, 128:128 + T],
                            rhs=wdc_sb[rows, :],
                            start=False, stop=True,
                        )
                        o_sb = outsp.tile([128, C], dt.float32, tag=f"os{bi}")
                        nc.vector.tensor_scalar(
                            o_sb[:T, :], op_ps[:], disown_sb[:T, tt:tt + 1],
                            None, alu.mult)
                        nc.vector.tensor_add(
                            o_sb[:T, :], o_sb[:T, :], bias_sb[:T, :])
                        nc.sync.dma_start(
                            outs[bi][tt * T:(tt + 1) * T, :], o_sb[:T, :])

    nc.compile()
    return nc


def _shared_weights(W_conc, W_disc, bias):
    wct2 = np.zeros((128, C), np.float32)
    wdt2 = np.zeros((128, C), np.float32)
    wct2[:64] = np.asarray(W_conc, np.float32).T  # WcT[i, o] = Wc[o, i]
    wct2[64:] = wct2[:64]
    wdt2[:64] = np.asarray(W_disc, np.float32).T
    wdt2[64:] = wdt2[:64]
    biasr = np.tile(np.asarray(bias, np.float32)[None, :], (128, 1))
    return wct2.astype(BF16), wdt2.astype(BF16), biasr


_NC_CACHE = {}


def _caps_needed(edge_index, n, n_cores, tile, split):
    """Max per-tile chunk counts for the A/B table split (no self loops)."""
    src0 = np.asarray(edge_index[0]).astype(np.int64)
    tgt0 = np.asarray(edge_index[1]).astype(np.int64)
    order = np.argsort(tgt0, kind="stable")
    tgt_s, src_s = tgt0[order], src0[order]
    starts = np.searchsorted(tgt_s, np.arange(0, n + 1, tile))
    na = np.add.reduceat((src_s < split).astype(np.int64), starts[:-1])
    tot = np.diff(starts)
    maxa = int(na.max())
    maxb = int((tot - na).max())
    return -(-maxa // CHUNK), -(-maxb // CHUNK)


def _make_in_maps(x, edge_index, f_disc_orig, fluxes, W_conc, W_disc, bias,
                  cfg):
    shared, cores = prep(x, edge_index, f_disc_orig, fluxes, cfg)
    wct2, wdt2, biasr = _shared_weights(W_conc, W_disc, bias)
    in_maps = []
    for core in range(cfg.n_cores):
        m = dict(shared)
        m.update(cores[core])
        m["wct2"] = wct2
        m["wdt2"] = wdt2
        m["biasr"] = biasr
        in_maps.append(m)
    return in_maps


def _run(inputs, trace=False):
    from concourse.bass_utils import run_bass_kernel_spmd

    x = np.asarray(inputs["x"], np.float32)
    n = x.shape[1]
    capa, capb = _caps_needed(inputs["edge_index"], n, N_CORES, TILE, SPLIT)
    cfg = Cfg(n_nodes=n, n_cores=N_CORES, tile=TILE, split=SPLIT,
              capa=max(capa, 2), capb=max(capb, 2))
    in_maps = _make_in_maps(
        x, inputs["edge_index"], inputs["f_disc_orig"], inputs["fluxes"],
        inputs["W_conc"], inputs["W_disc"], inputs["bias"], cfg)

    if cfg not in _NC_CACHE:
        _NC_CACHE[cfg] = build_nc(cfg)
    nc = _NC_CACHE[cfg]

    res = run_bass_kernel_spmd(nc, in_maps, list(range(cfg.n_cores)),
                               trace=trace)
    out = np.zeros((BATCH, n, C), np.float32)
    npc = cfg.nodes_per_core
    for core in range(cfg.n_cores):
        out[0, core * npc:(core + 1) * npc] = res.results[core]["out0"]
        out[1, core * npc:(core + 1) * npc] = res.results[core]["out1"]
    return out, res


def kernel(x, edge_index, f_disc_orig, fluxes, W_conc, W_disc, bias):
    out, _ = _run(dict(x=x, edge_index=edge_index, f_disc_orig=f_disc_orig,
                       fluxes=fluxes, W_conc=W_conc, W_disc=W_disc, bias=bias))
    return out


def profile_run(inputs):
    out, res = _run(inputs, trace=True)
    return res.exec_time_ns


# revision 23
# speedup vs baseline: 1.4649x; 1.0149x over previous
"""Trainium2 Bass kernel for nn_ReaReaConv (GCN-style message passing with
dynamic edge gating).

Math (per batch b):
    deg[n]   = in-degree(n) + 1 (self loop);  dis = rsqrt(deg)
    f_e      = keep*fdo + (1-keep)*(1-fdo), keep = sigmoid(2*flux[src]*flux[tgt])
    out[t]   = dis_t * ( T[t] @ Wc^T + V[t] @ (Wd-Wc)^T ) + bias
    T[t]     = sum_{e->t} dis_src * x[src_e] + dis_t * x[t]   (self loop f=0)
    V[t]     = sum_{e->t} dis_src * f_e * x[src_e]

Sharding: each of the 8 cores owns N/8 target nodes (tiles of 125). Host sorts
edges by target tile, prescales the gather table by dis_src and casts to bf16
(index/layout/elementwise-prescale only; per-edge math runs on device).

v4 design notes:
 - Gather table rows are batch-PAIR-interleaved ([x0[c],x1[c]] pairs) so the
   on-device f-scale multiply qualifies for the DVE 2x 16-bit mode (the f
   broadcast lands mid-dim, innermost is the contiguous batch pair).
 - One-hot is built t-major (col = t*ct + c) against a host-built wide iota so
   every operand of the is_equal has innermost stride 1 -> DVE 2x mode.
 - Self-loop contributions are streamed contiguously (no gather descriptors)
   and added to T as a [125, 128] DVE add.
 - Per 128-edge chunk: ONE matmul, lhsT = one-hot [128, 125], rhs = [y | f*y]
   [128, 256] accumulating PSUM [125, 256] = [T | V]. Epilogue PE-transposes
   (deinterleaving batch pairs for free) and projects with Wc / (Wd-Wc).
"""

from dataclasses import dataclass

import ml_dtypes
import numpy as np

BF16 = ml_dtypes.bfloat16

# -------------------- problem constants --------------------
N_NODES = 50000
N_EDGES = 1600000
BATCH = 2
C = 64
N_CORES = 8
TILE = 125           # target nodes per tile (one-hot width)
CHUNK = 128          # edges per matmul chunk (PE contraction)
SPLIT = 32768        # gather-table split (int16 signed index limit)


@dataclass(frozen=True)
class Cfg:
    n_nodes: int
    n_cores: int
    tile: int
    split: int
    capa: int  # chunks per tile from table A (src < split)
    capb: int  # chunks per tile from table B

    @property
    def nodes_per_core(self):
        return self.n_nodes // self.n_cores

    @property
    def ntc(self):  # tiles per core
        return self.nodes_per_core // self.tile

    @property
    def ct(self):
        return self.capa + self.capb

    @property
    def ctn(self):
        return self.ntc * self.ct

    @property
    def na(self):
        return min(self.split, self.n_nodes)

    @property
    def nb(self):
        return self.n_nodes - self.na


# -------------------- host prep (indices / layout / dtype cast) ------------

def _wrap16(idx_flat):
    """dma_gather index layout: [128, n/16] int16, idx[p, s] = flat[s*16+p],
    replicated across the 8 gpsimd cores (partition blocks of 16)."""
    n = len(idx_flat)
    assert n % 16 == 0
    w = np.asarray(idx_flat, np.int16).reshape(n // 16, 16).T  # [16, n/16]
    return np.tile(w, (8, 1))  # [128, n/16]


def prep(x, edge_index, f_disc_orig, fluxes, cfg: Cfg):
    """Returns (shared dict, list of per-core dicts). Index/layout/cast and
    elementwise dis prescale only — per-edge math runs on device."""
    n = cfg.n_nodes
    src0 = np.asarray(edge_index[0]).astype(np.int64)
    tgt0 = np.asarray(edge_index[1]).astype(np.int64)
    x = np.asarray(x, np.float32)
    fdo_in = np.asarray(f_disc_orig, np.float32)
    fluxes = np.asarray(fluxes, np.float32)

    deg = (np.bincount(tgt0, minlength=n) + 1).astype(np.float32)  # int-valued
    dis = 1.0 / np.sqrt(deg)

    # pair-interleaved, dis-prescaled bf16 gather table:
    # row n = [x0[n,0], x1[n,0], x0[n,1], x1[n,1], ...] * dis_n
    xi = np.empty((n, 2 * C), np.float32)
    xi[:, 0::2] = x[0] * dis[:, None]
    xi[:, 1::2] = x[1] * dis[:, None]
    xi = xi.astype(BF16)

    per_edge_all = np.stack([
        fdo_in,
        fluxes[0][src0],
        fluxes[1][src0],
        fluxes[0][tgt0],
        fluxes[1][tgt0],
    ])  # [5, E]: fdo, fs0, fs1, ft0, ft1   (no self loops)

    perm = np.argsort(tgt0, kind="stable")
    src_s = src0[perm]
    tgt_s = tgt0[perm]
    per_edge = per_edge_all[:, perm]

    tile_starts = np.searchsorted(tgt_s, np.arange(0, n + 1, cfg.tile))
    is_a = src_s < cfg.split

    ct, ntc, ctn = cfg.ct, cfg.ntc, cfg.ctn
    T = cfg.tile

    shared = {
        "xpa": np.ascontiguousarray(xi[: cfg.na]),
        "xpb": np.ascontiguousarray(xi[cfg.na:]),
        # wide iota, t-major: col t*ct + c = t
        "iotaw": np.tile(np.repeat(np.arange(T, dtype=np.float32), ct),
                         (128, 1)).astype(BF16),
        "ident": np.eye(128, dtype=np.float32).astype(BF16),
    }

    names = ["fdo", "fs0", "fs1", "ft0", "ft1"]
    npc = cfg.nodes_per_core
    cores = []
    for core in range(cfg.n_cores):
        tl_all = np.full((128, ctn), -1.0, np.float32)
        pe_all = np.zeros((5, 128, ctn), np.float32)
        idx16 = np.zeros((128, ctn * 8), np.int16)
        degown = np.ones((128, ntc), np.float32)
        for tt in range(ntc):
            t = core * ntc + tt
            t0 = t * cfg.tile
            s, e = tile_starts[t], tile_starts[t + 1]
            sel_a = np.nonzero(is_a[s:e])[0] + s
            sel_b = np.nonzero(~is_a[s:e])[0] + s
            nA, nB = len(sel_a), len(sel_b)
            assert nA <= cfg.capa * CHUNK, (t, nA, cfg.capa * CHUNK)
            assert nB <= cfg.capb * CHUNK, (t, nB, cfg.capb * CHUNK)

            ids = np.zeros(ct * CHUNK, np.int64)
            tl = np.full(ct * CHUNK, -1.0, np.float32)
            pe = np.zeros((5, ct * CHUNK), np.float32)
            off = cfg.capa * CHUNK
            ids[:nA] = src_s[sel_a]
            ids[off:off + nB] = src_s[sel_b] - cfg.na
            tl[:nA] = tgt_s[sel_a] - t0
            tl[off:off + nB] = tgt_s[sel_b] - t0
            pe[:, :nA] = per_edge[:, sel_a]
            pe[:, off:off + nB] = per_edge[:, sel_b]

            # chunk-transposed layout: element (p, c) = edge[c*128+p]
            cols = slice(tt * ct, (tt + 1) * ct)
            tl_all[:, cols] = tl.reshape(ct, CHUNK).T
            for j in range(5):
                pe_all[j][:, cols] = pe[j].reshape(ct, CHUNK).T

            islice = idx16[:, tt * ct * 8:(tt + 1) * ct * 8]
            islice[:, : cfg.capa * 8] = _wrap16(ids[:off])
            islice[:, cfg.capa * 8:] = _wrap16(ids[off:])

            degown[:cfg.tile, tt] = deg[t0:t0 + cfg.tile]
        d = {"tl": tl_all.astype(BF16), "idx16": idx16, "degown": degown,
             "yloop": np.ascontiguousarray(xi[core * npc:(core + 1) * npc])}
        for j, nm in enumerate(names):
            d[nm] = np.ascontiguousarray(pe_all[j])
        cores.append(d)
    return shared, cores


# -------------------- device program --------------------

def build_nc(cfg: Cfg):
    import concourse.bass as bass
    import concourse.tile as tile
    from concourse import bacc, mybir

    dt = mybir.dt
    act = mybir.ActivationFunctionType
    alu = mybir.AluOpType

    ct, capa, capb = cfg.ct, cfg.capa, cfg.capb
    ntc, T, ctn = cfg.ntc, cfg.tile, cfg.ctn

    nc = bacc.Bacc("TRN2", target_bir_lowering=False, debug=False,
                   num_swdge_queues=4)

    xpa = nc.dram_tensor("xpa", [cfg.na, 2 * C], dt.bfloat16, kind="ExternalInput")
    xpb = nc.dram_tensor("xpb", [cfg.nb, 2 * C], dt.bfloat16, kind="ExternalInput")
    yl_d = nc.dram_tensor("yloop", [ntc * T, 2 * C], dt.bfloat16, kind="ExternalInput")
    tl_d = nc.dram_tensor("tl", [128, ctn], dt.bfloat16, kind="ExternalInput")
    fdo_d = nc.dram_tensor("fdo", [128, ctn], dt.float32, kind="ExternalInput")
    fs0_d = nc.dram_tensor("fs0", [128, ctn], dt.float32, kind="ExternalInput")
    fs1_d = nc.dram_tensor("fs1", [128, ctn], dt.float32, kind="ExternalInput")
    ft0_d = nc.dram_tensor("ft0", [128, ctn], dt.float32, kind="ExternalInput")
    ft1_d = nc.dram_tensor("ft1", [128, ctn], dt.float32, kind="ExternalInput")
    idx16_d = nc.dram_tensor("idx16", [128, ctn * 8], dt.int16, kind="ExternalInput")
    degown_d = nc.dram_tensor("degown", [128, ntc], dt.float32, kind="ExternalInput")
    iotaw_d = nc.dram_tensor("iotaw", [128, T * ct], dt.bfloat16, kind="ExternalInput")
    ident_d = nc.dram_tensor("ident", [128, 128], dt.bfloat16, kind="ExternalInput")
    wct_d = nc.dram_tensor("wct2", [128, C], dt.bfloat16, kind="ExternalInput")
    wdt_d = nc.dram_tensor("wdt2", [128, C], dt.bfloat16, kind="ExternalInput")
    bias_d = nc.dram_tensor("biasr", [128, C], dt.float32, kind="ExternalInput")
    out0 = nc.dram_tensor("out0", [ntc * T, C], dt.float32, kind="ExternalOutput")
    out1 = nc.dram_tensor("out1", [ntc * T, C], dt.float32, kind="ExternalOutput")
    outs = [out0, out1]

    with tile.TileContext(nc) as tc:
        with (
            tc.tile_pool(name="const", bufs=1) as constp,
            tc.tile_pool(name="res", bufs=1) as resp,
        ):
            iotaw_sb = constp.tile([128, T * ct], dt.bfloat16)
            nc.sync.dma_start(iotaw_sb[:], iotaw_d[:, :])
            ident_sb = constp.tile([128, 128], dt.bfloat16)
            nc.sync.dma_start(ident_sb[:], ident_d[:, :])
            bias_sb = constp.tile([128, C], dt.float32)
            nc.sync.dma_start(bias_sb[:], bias_d[:, :])
            wct_sb = constp.tile([128, C], dt.bfloat16)
            nc.sync.dma_start(wct_sb[:], wct_d[:, :])
            wdt_sb = constp.tile([128, C], dt.bfloat16)
            nc.sync.dma_start(wdt_sb[:], wdt_d[:, :])
            # wdc = Wd - Wc (projection uses T*Wc + V*(Wd-Wc))
            wdc_sb = constp.tile([128, C], dt.bfloat16)
            nc.vector.tensor_tensor(wdc_sb[:], wdt_sb[:], wct_sb[:],
                                    alu.subtract)

            # resident per-core data
            tl_sb = resp.tile([128, ctn], dt.bfloat16)
            nc.sync.dma_start(tl_sb[:], tl_d[:, :])
            idx_sb = resp.tile([128, ctn * 8], dt.int16)
            nc.sync.dma_start(idx_sb[:], idx16_d[:, :])
            # f for both batches, interleaved [128, ctn, 2] bf16
            fcat_sb = resp.tile([128, ctn * 2], dt.bfloat16)
            fcat3 = fcat_sb[:].rearrange("p (n b) -> p n b", b=2)

            # dis for own target nodes (fp32, output scale)
            disown_sb = resp.tile([128, ntc], dt.float32)
            nc.sync.dma_start(disown_sb[:], degown_d[:, :])
            nc.vector.reciprocal(disown_sb[:], disown_sb[:])
            nc.scalar.activation(disown_sb[:], disown_sb[:], act.Sqrt)

            # ---- prepass: f0/f1 from flux/fdo ----
            with tc.tile_pool(name="pp", bufs=1) as ppp:
                fdo_sb = ppp.tile([128, ctn], dt.float32)
                nc.sync.dma_start(fdo_sb[:], fdo_d[:, :])
                c1 = ppp.tile([128, ctn], dt.float32)
                nc.vector.tensor_scalar(
                    c1[:], fdo_sb[:], 2.0, -1.0, alu.mult, alu.add)
                c0 = ppp.tile([128, ctn], dt.float32)
                nc.vector.tensor_scalar(
                    c0[:], fdo_sb[:], -1.0, 1.0, alu.mult, alu.add)
                for b, (fsd, ftd) in enumerate(((fs0_d, ft0_d), (fs1_d, ft1_d))):
                    fs_sb = ppp.tile([128, ctn], dt.float32, tag="fs")
                    nc.sync.dma_start(fs_sb[:], fsd[:, :])
                    ft_sb = ppp.tile([128, ctn], dt.float32, tag="ft")
                    nc.sync.dma_start(ft_sb[:], ftd[:, :])
                    nc.vector.tensor_mul(fs_sb[:], fs_sb[:], ft_sb[:])
                    nc.scalar.activation(
                        ft_sb[:], fs_sb[:], act.Sigmoid, scale=2.0)
                    nc.vector.tensor_mul(fs_sb[:], ft_sb[:], c1[:])
                    nc.vector.tensor_add(fcat3[:, :, b], fs_sb[:], c0[:])


            # ---- main loop over node tiles ----
            with (
                tc.tile_pool(name="xg", bufs=4) as xgp,
                tc.tile_pool(name="yl", bufs=3) as ylp,
                tc.tile_pool(name="oe", bufs=3) as oep,
                tc.tile_pool(name="tvs", bufs=2) as tvsp,
                tc.tile_pool(name="trs", bufs=2) as trsp,
                tc.tile_pool(name="outp", bufs=2) as outsp,
                tc.tile_pool(name="ps_tv", bufs=2, space="PSUM") as pstv,
                tc.tile_pool(name="ps_tr", bufs=2, space="PSUM") as pstr,
                tc.tile_pool(name="ps_o", bufs=2, space="PSUM") as pso,
            ):
                # balanced queue split: ~ct/4 chunks per SWDGE queue so the
                # per-tile gather latency tracks ct/4, not max(capa/2, capb)
                gathers = []
                pos = 0
                for q in range(4):
                    take = (ct + 3 - q) // 4
                    while take > 0:
                        if pos < capa:
                            nn = min(take, capa - pos)
                            gathers.append((pos, pos + nn, True, q))
                        else:
                            nn = min(take, ct - pos)
                            gathers.append((pos, pos + nn, False, q))
                        pos += nn
                        take -= nn

                for tt in range(ntc):
                    ib = tt * ct * 8
                    # gathered rows (first half) | f-scaled rows (second half)
                    xg = xgp.tile([128, 2 * ct * 2 * C], dt.bfloat16, tag="xg")
                    xg3 = xg[:].rearrange("p (c r) -> p c r", r=2 * C)
                    for (c0_, c1_, is_a_, q) in gathers:
                        nch = c1_ - c0_
                        nc.gpsimd.dma_gather(
                            xg3[:, c0_:c1_],
                            (xpa if is_a_ else xpb)[:, :],
                            idx_sb[:, ib + c0_ * 8: ib + c1_ * 8],
                            nch * CHUNK, nch * CHUNK, 2 * C,
                            single_packet=False, queue_num=q,
                        )

                    # self-loop rows for this tile (contiguous stream)
                    yl_sb = ylp.tile([128, 2 * C], dt.bfloat16, tag="yl")
                    nc.sync.dma_start(
                        yl_sb[:T, :], yl_d[tt * T:(tt + 1) * T, :])

                    # one-hot, t-major (col = t*ct + c); table is
                    # dis_src-prescaled so values are pure 0/1
                    o_all = oep.tile([128, T * ct], dt.bfloat16, tag="oe")
                    o3 = o_all[:].rearrange("p (t c) -> p t c", c=ct)
                    tl_cols = tl_sb[:, tt * ct:(tt + 1) * ct].unsqueeze(1)
                    nc.vector.tensor_tensor(
                        o3,
                        tl_cols.to_broadcast([128, T, ct]),
                        iotaw_sb[:].rearrange("p (t c) -> p t c", c=ct),
                        alu.is_equal,
                    )

                    # second half = f * first half (per batch, pair-interleaved)
                    xh4 = xg[:, 0:ct * 2 * C].rearrange(
                        "p (c f b) -> p c f b", f=C, b=2)
                    fxh4 = xg[:, ct * 2 * C:2 * ct * 2 * C].rearrange(
                        "p (c f b) -> p c f b", f=C, b=2)
                    fcols = fcat3[:, tt * ct:(tt + 1) * ct, :].unsqueeze(2)
                    nc.vector.tensor_tensor(
                        fxh4, xh4, fcols.to_broadcast([128, ct, C, 2]),
                        alu.mult)

                    # [T | V] accumulation: one matmul per chunk, 256-wide rhs
                    tv_ps = pstv.tile([T, 4 * C], dt.float32, tag="tv")
                    xgv = xg[:].rearrange("p (h w) -> p h w", h=2)
                    for c in range(ct):
                        nc.tensor.matmul(
                            out=tv_ps[:],
                            lhsT=o3[:, :, c],
                            rhs=xgv[:, :, c * 2 * C:(c + 1) * 2 * C],
                            start=(c == 0), stop=(c == ct - 1),
                        )

                    # epilogue: deinterleaving bf16 copy (pair-interleaved ->
                    # batch-major) + self-loop add, transposes to [ch, t],
                    # project
                    tv_sb = tvsp.tile([T, 4 * C], dt.bfloat16, tag="tv_sb")
                    nc.scalar.activation(
                        tv_sb[:].rearrange("p (g b f) -> p g b f",
                                           g=2, b=2, f=C),
                        tv_ps[:].rearrange("p (g f b) -> p g b f",
                                           g=2, f=C, b=2),
                        act.Copy)
                    nc.vector.tensor_add(
                        tv_sb[:, 0:2 * C].rearrange("p (b f) -> p b f", b=2),
                        tv_sb[:, 0:2 * C].rearrange("p (b f) -> p b f", b=2),
                        yl_sb[:T, :].rearrange("p (f b) -> p b f", b=2))
                    tr_ps = pstr.tile([128, 256], dt.bfloat16, tag="tr")
                    nc.tensor.transpose(
                        tr_ps[:, 0:T], tv_sb[:, 0:2 * C], ident_sb[:T, :T])
                    nc.tensor.transpose(
                        tr_ps[:, 128:128 + T], tv_sb[:, 2 * C:4 * C],
                        ident_sb[:T, :T])
                    tr_sb = trsp.tile([128, 256], dt.bfloat16, tag="tr_sb")
                    nc.vector.tensor_copy(
                        out=tr_sb[:, 0:T], in_=tr_ps[:, 0:T])
                    nc.scalar.activation(
                        tr_sb[:, 128:128 + T], tr_ps[:, 128:128 + T], act.Copy)

                    for bi in range(2):
                        rows = slice(64 * bi, 64 * bi + 64)
                        op_ps = pso.tile([T, C], dt.float32, tag=f"op{bi}")
                        nc.tensor.matmul(
                            out=op_ps[:], lhsT=tr_sb[rows, 0:T],
                            rhs=wct_sb[rows, :],
                            start=True, stop=False,
                        )
                        nc.tensor.matmul(
                            out=op_ps[:], lhsT=tr_sb[rows, 128:128 + T],
                            rhs=wdc_sb[rows, :],
                            start=False, stop=True,
                        )
                        o_sb = outsp.tile([128, C], dt.float32, tag=f"os{bi}")
                        nc.scalar.activation(
                            o_sb[:T, :], op_ps[:], act.Copy,
                            scale=disown_sb[:T, tt:tt + 1])
                        nc.vector.tensor_add(
                            o_sb[:T, :], o_sb[:T, :], bias_sb[:T, :])
                        nc.sync.dma_start(
                            outs[bi][tt * T:(tt + 1) * T, :], o_sb[:T, :])

    nc.compile()
    return nc


def _shared_weights(W_conc, W_disc, bias):
    wct2 = np.zeros((128, C), np.float32)
    wdt2 = np.zeros((128, C), np.float32)
    wct2[:64] = np.asarray(W_conc, np.float32).T  # WcT[i, o] = Wc[o, i]
    wct2[64:] = wct2[:64]
    wdt2[:64] = np.asarray(W_disc, np.float32).T
    wdt2[64:] = wdt2[:64]
    biasr = np.tile(np.asarray(bias, np.float32)[None, :], (128, 1))
    return wct2.astype(BF16), wdt2.astype(BF16), biasr


_NC_CACHE = {}


def _caps_needed(edge_index, n, n_cores, tile, split):
    """Max per-tile chunk counts for the A/B table split (no self loops)."""
    src0 = np.asarray(edge_index[0]).astype(np.int64)
    tgt0 = np.asarray(edge_index[1]).astype(np.int64)
    order = np.argsort(tgt0, kind="stable")
    tgt_s, src_s = tgt0[order], src0[order]
    starts = np.searchsorted(tgt_s, np.arange(0, n + 1, tile))
    na = np.add.reduceat((src_s < split).astype(np.int64), starts[:-1])
    tot = np.diff(starts)
    maxa = int(na.max())
    maxb = int((tot - na).max())
    return -(-maxa // CHUNK), -(-maxb // CHUNK)


def _make_in_maps(x, edge_index, f_disc_orig, fluxes, W_conc, W_disc, bias,
                  cfg):
    shared, cores = prep(x, edge_index, f_disc_orig, fluxes, cfg)
    wct2, wdt2, biasr = _shared_weights(W_conc, W_disc, bias)
    in_maps = []
    for core in range(cfg.n_cores):
        m = dict(shared)
        m.update(cores[core])
        m["wct2"] = wct2
        m["wdt2"] = wdt2
        m["biasr"] = biasr
        in_maps.append(m)
    return in_maps


def _run(inputs, trace=False):
    from concourse.bass_utils import run_bass_kernel_spmd

    x = np.asarray(inputs["x"], np.float32)
    n = x.shape[1]
    capa, capb = _caps_needed(inputs["edge_index"], n, N_CORES, TILE, SPLIT)
    cfg = Cfg(n_nodes=n, n_cores=N_CORES, tile=TILE, split=SPLIT,
              capa=max(capa, 2), capb=max(capb, 2))
    in_maps = _make_in_maps(
        x, inputs["edge_index"], inputs["f_disc_orig"], inputs["fluxes"],
        inputs["W_conc"], inputs["W_disc"], inputs["bias"], cfg)

    if cfg not in _NC_CACHE:
        _NC_CACHE[cfg] = build_nc(cfg)
    nc = _NC_CACHE[cfg]

    res = run_bass_kernel_spmd(nc, in_maps, list(range(cfg.n_cores)),
                               trace=trace)
    out = np.zeros((BATCH, n, C), np.float32)
    npc = cfg.nodes_per_core
    for core in range(cfg.n_cores):
        out[0, core * npc:(core + 1) * npc] = res.results[core]["out0"]
        out[1, core * npc:(core + 1) * npc] = res.results[core]["out1"]
    return out, res


def kernel(x, edge_index, f_disc_orig, fluxes, W_conc, W_disc, bias):
    out, _ = _run(dict(x=x, edge_index=edge_index, f_disc_orig=f_disc_orig,
                       fluxes=fluxes, W_conc=W_conc, W_disc=W_disc, bias=bias))
    return out


def profile_run(inputs):
    out, res = _run(inputs, trace=True)
    return res.exec_time_ns
